# revision 13
# baseline (speedup 1.0000x reference)
"""MemoryBank kernel for Trainium2, 8 NeuronCores (SPMD).

Reference computation:
    s        = rowsum(W)                       # [B]
    update   = (W/s[:,None]).T @ X             # [M, F]
    mem_new  = memory + update
    scores   = Q @ mem_new.T                   # [B, M]
    attn     = softmax(scores, axis=1)
    retrieved = attn @ mem_new                 # [B, F]
    conf     = max(attn, axis=1)               # [B]

Sharding: memory_size axis. Core c owns slots [c*4096, (c+1)*4096).
Each core reads its column slice of W (64 MB) EXACTLY ONCE from HBM
(the memory roofline), computing per-batch-group partial rowsums on the
fly; tiny [512]-float AllReduces make the rowsums global, after which
the resident W tiles feed the update matmul (update.T accumulated
entirely in PSUM over all 32 batch chunks).

Attention runs transposed (scoresT[slot, batch] = mem_newT.T @ QT) so
softmax stats never need a partition-axis reduction on the hot path:
  - exp() with NO max subtraction (scores |max| ~ 48 << 88, fp32-safe);
    softmax denominator Z comes for free as a ones-column in the
    retrieved matmul; confidence = maxE / Z with maxE tracked by a
    bf16 elementwise-max accumulator (partition-reduced once at end
    via PE transposes).
  - retrieved partials: R[b, f+1] = sum_j E[j,b] * [mem_new | 1][j, f]
Host combines: sums R/Z partials over cores, maxes maxE, concatenates
mem_new slices. Only device collective: the rowsum AllReduces.

Precision: W/X/E/mem_new(bf16) paths ~0.4% RMS; scores in fp32r
(~2.5e-4) since exp amplifies score error.
"""
import sys

if "/opt/trn_rl_repo" not in sys.path:
    sys.path.insert(0, "/opt/trn_rl_repo")

import numpy as np

import concourse.bacc as bacc
import concourse.mybir as mybir
import concourse.tile as tile
import concourse.masks as masks
from concourse.bass_utils import run_bass_kernel_spmd

N_CORES = 8
B = 4096          # batch
F = 128           # feature dim
M = 32768         # memory slots (global)
MS = M // N_CORES  # slots per core = 4096
NB = B // 128      # batch chunks of 128 = 32
NG = 8             # batch groups (of 512 rows) for rowsum AllReduces
GK = NB // NG      # chunks per group = 4
SC = MS // 128     # slot chunks per core = 32
AG = 8             # attention batch groups of 512
F32 = mybir.dt.float32
F32R = mybir.dt.float32r
BF16 = mybir.dt.bfloat16

_CACHED_NC = None


def _build():
    nc = bacc.Bacc("TRN2", target_bir_lowering=False, debug=False,
                   num_devices=N_CORES)
    w = nc.dram_tensor("w", [B, MS], F32, kind="ExternalInput").ap()
    qt = nc.dram_tensor("qt", [F, B], F32, kind="ExternalInput").ap()
    x = nc.dram_tensor("x", [B, F], F32, kind="ExternalInput").ap()
    mt = nc.dram_tensor("mt", [F, MS], F32, kind="ExternalInput").ap()

    mem_newt_o = nc.dram_tensor("mem_newt", [F, MS], F32,
                                kind="ExternalOutput").ap()
    r_o = nc.dram_tensor("r_out", [AG, 128, 4 * 129], F32,
                         kind="ExternalOutput").ap()
    maxe_o = nc.dram_tensor("maxe", [128, NB], F32,
                            kind="ExternalOutput").ap()

    rg = [list(range(N_CORES))]

    with tile.TileContext(nc) as tc:
        with (
            tc.tile_pool(name="const", bufs=1) as const,
            tc.tile_pool(name="wpool", bufs=8) as wpool,
            tc.tile_pool(name="spool", bufs=4) as spool,
            tc.tile_pool(name="dram", bufs=16, space="DRAM") as dram,
            tc.tile_pool(name="epool", bufs=3) as epool,
            tc.tile_pool(name="opool", bufs=2) as opool,
        ):
            # ---------- persistent tiles ----------
            ident = const.tile([128, 128], F32)
            masks.make_identity(nc, ident[:])

            # x rearranged to [128, 32*128]: chunk i at cols i*128..,
            # partition p = batch row i*128+p. bf16 (x/s later in bf16).
            xsb = const.tile([128, NB * F], BF16)
            nc.gpsimd.dma_start(
                out=xsb[:].rearrange("p (c f) -> p c f", c=NB),
                in_=x.rearrange("(c p) f -> p c f", p=128))
            xs = const.tile([128, NB * F], BF16)   # x scaled by 1/rowsum

            qtr = const.tile([F, B], F32R)
            nc.sync.dma_start(out=qtr[:], in_=qt[:, :].bitcast(F32R))
            mtt = const.tile([F, MS], F32)
            nc.sync.dma_start(out=mtt[:], in_=mt[:, :])

            mem_newt_r = const.tile([F, MS], F32R)   # memory+update, f32r
            mnbf = const.tile([F, MS], BF16)         # bf16 copy (for transp.)
            mnb = const.tile([128, SC * 129], BF16)  # [mem_new | 1] by chunk
            macc = const.tile([128, B], BF16)        # running max of E
            nc.vector.memset(macc[:], 0.0)
            nc.gpsimd.memset(mnb[:], 1.0)
            mef = const.tile([128, B], F32)          # macc as f32 for transp.
            mesb = opool.tile([128, NB], F32)
            nbias = const.tile([128, 1], F32)        # exp shift
            nc.vector.memset(nbias[:], -60.0)

            # ---------- write phase ----------
            with tc.tile_pool(name="psum_u", bufs=1, space="PSUM") as up:
                psum_u = up.tile([F, MS], F32)
                wtiles = {}
                for g in range(NG):
                    sp = spool.tile([128, GK], F32)
                    for k in range(GK):
                        i = g * GK + k
                        wt = wpool.tile([128, MS], BF16, tag="wt")
                        nc.gpsimd.dma_start(
                            out=wt[:], in_=w[i * 128:(i + 1) * 128, :])
                        nc.vector.reduce_sum(out=sp[:, k:k + 1], in_=wt[:],
                                             axis=mybir.AxisListType.X)
                        wtiles[i] = wt
                    cc_in = dram.tile([512], F32, tag="cc_in")
                    cc_out = dram.tile([512], F32, tag="cc_out")
                    nc.sync.dma_start(
                        out=cc_in[:].rearrange("(k p) -> p k", p=128),
                        in_=sp[:])
                    nc.gpsimd.collective_compute(
                        "AllReduce", mybir.AluOpType.add, replica_groups=rg,
                        ins=[cc_in[:]], outs=[cc_out[:]])
                    rsg = spool.tile([128, GK], F32)
                    nc.sync.dma_start(
                        out=rsg[:],
                        in_=cc_out[:].rearrange("(k p) -> p k", p=128))
                    rc = spool.tile([128, GK], F32)
                    nc.vector.reciprocal(rc[:], rsg[:])
                    for k in range(GK):
                        i = g * GK + k
                        nc.vector.tensor_scalar_mul(
                            xs[:, i * F:(i + 1) * F],
                            xsb[:, i * F:(i + 1) * F], rc[:, k:k + 1])
                        wt = wtiles.pop(i)
                        for n in range(MS // 512):
                            nc.tensor.matmul(
                                psum_u[:, n * 512:(n + 1) * 512],
                                lhsT=xs[:, i * F:(i + 1) * F],
                                rhs=wt[:, n * 512:(n + 1) * 512],
                                start=(i == 0), stop=(i == NB - 1))

                # mem_newT = update.T + memory.T, rounded to f32r
                nc.vector.tensor_add(mem_newt_r[:], psum_u[:], mtt[:])

            nc.sync.dma_start(out=mem_newt_o[:, :],
                              in_=mem_newt_r[:].bitcast(F32))
            nc.scalar.activation(mnbf[:], mem_newt_r[:].bitcast(F32),
                                 mybir.ActivationFunctionType.Copy)

            # ---------- attention phase ----------
            with (
                tc.tile_pool(name="psum_s", bufs=2, space="PSUM") as sps,
                tc.tile_pool(name="psum_r", bufs=1, space="PSUM") as rps,
                tc.tile_pool(name="psum_m", bufs=2, space="PSUM") as mps,
            ):
                # mnb: [mem_new | 1] per slot chunk, bf16 (via PE transpose)
                for t in range(SC):
                    pt = mps.tile([128, F], F32, tag="pt")
                    nc.tensor.transpose(
                        pt[:], mem_newt_r[:, t * 128:(t + 1) * 128]
                        .bitcast(F32), ident[:])
                    nc.scalar.activation(mnb[:, t * 129:t * 129 + 128], pt[:],
                                         mybir.ActivationFunctionType.Copy)

                # Keep PE transpose-mode ops strictly before the attention
                # Ldweights/Matmult stream (HW transpose-mode interleave
                # corrupts the weight path).
                tc.no_sync_barrier()

                for g in range(AG):
                    # One PSUM bank per accumulator: start=True clears the
                    # whole bank, so co-resident tiles would be erased.
                    rts = []
                    for bt in range(4):
                        rtile = rps.tile([128, 129], F32, tag=f"r{bt}")
                        rts.append(rtile)
                    for t in range(SC):
                        st = sps.tile([128, 512], F32, tag="st")
                        nc.tensor.matmul(
                            st[:], lhsT=mem_newt_r[:, t * 128:(t + 1) * 128],
                            rhs=qtr[:, g * 512:(g + 1) * 512],
                            start=True, stop=True)
                        # exp(s - 60): constant shift cancels in softmax and
                        # in maxE/Z; keeps exp(max~89) inside fp32/bf16 range.
                        et = epool.tile([128, 512], BF16, tag="et")
                        nc.scalar.activation(et[:], st[:],
                                             mybir.ActivationFunctionType.Exp,
                                             bias=nbias[:])
                        nc.vector.tensor_max(macc[:, g * 512:(g + 1) * 512],
                                             macc[:, g * 512:(g + 1) * 512],
                                             et[:])
                        for bt in range(4):
                            nc.tensor.matmul(
                                rts[bt][:, :],
                                lhsT=et[:, bt * 128:(bt + 1) * 128],
                                rhs=mnb[:, t * 129:(t + 1) * 129],
                                start=(t == 0), stop=(t == SC - 1))
                    rsb = opool.tile([128, 4 * 129], F32, tag="rsb")
                    for bt in range(4):
                        nc.scalar.activation(
                            rsb[:, bt * 129:(bt + 1) * 129], rts[bt][:],
                            mybir.ActivationFunctionType.Copy)
                    nc.sync.dma_start(out=r_o[g, :, :], in_=rsb[:])

                # confidence: partition-reduce macc via PE transposes
                tc.no_sync_barrier()
                nc.vector.tensor_copy(mef[:], macc[:])
                for t in range(NB):
                    pt2 = mps.tile([128, 128], F32, tag="pt")
                    nc.tensor.transpose(pt2[:], mef[:, t * 128:(t + 1) * 128],
                                        ident[:])
                    nc.vector.reduce_max(out=mesb[:, t:t + 1], in_=pt2[:],
                                         axis=mybir.AxisListType.X)
                nc.sync.dma_start(out=maxe_o[:, :], in_=mesb[:])

    nc.compile()
    return nc


def kernel(memory, input_data, write_weights, query):
    global _CACHED_NC
    memory = np.asarray(memory, dtype=np.float32)
    input_data = np.asarray(input_data, dtype=np.float32)
    write_weights = np.asarray(write_weights, dtype=np.float32)
    query = np.asarray(query, dtype=np.float32)

    if _CACHED_NC is None:
        _CACHED_NC = _build()
    nc = _CACHED_NC

    qt = np.ascontiguousarray(query.T)
    in_maps = []
    for c in range(N_CORES):
        in_maps.append({
            "w": np.ascontiguousarray(write_weights[:, c * MS:(c + 1) * MS]),
            "qt": qt,
            "x": input_data,
            "mt": np.ascontiguousarray(memory[c * MS:(c + 1) * MS, :].T),
        })

    res = run_bass_kernel_spmd(nc, in_maps, list(range(N_CORES))).results

    # ---------- host combine (cheap: ~2MB/core) ----------
    mem_new = np.concatenate(
        [res[c]["mem_newt"].T for c in range(N_CORES)], axis=0)

    R = np.zeros((B, 129), dtype=np.float64)
    maxe = np.zeros(B, dtype=np.float64)
    for c in range(N_CORES):
        r4 = res[c]["r_out"].reshape(AG, 128, 4, 129)  # [g, p, bt, col]
        R += r4.transpose(0, 2, 1, 3).reshape(B, 129)
        me = res[c]["maxe"].T.reshape(B)               # b = t*128 + p
        maxe = np.maximum(maxe, me)

    z = R[:, 128]
    retrieved = (R[:, :F] / z[:, None]).astype(np.float32)
    conf = (maxe / z).astype(np.float32)
    return retrieved, conf, np.ascontiguousarray(mem_new)


# revision 14
# speedup vs baseline: 1.1604x; 1.1604x over previous
"""MemoryBank kernel for Trainium2, 8 NeuronCores (SPMD).

Reference computation:
    s        = rowsum(W)                       # [B]
    update   = (W/s[:,None]).T @ X             # [M, F]
    mem_new  = memory + update
    scores   = Q @ mem_new.T                   # [B, M]
    attn     = softmax(scores, axis=1)
    retrieved = attn @ mem_new                 # [B, F]
    conf     = max(attn, axis=1)               # [B]

Sharding: memory_size axis. Core c owns slots [c*4096, (c+1)*4096).
Each core reads its column slice of W (64 MB) EXACTLY ONCE from HBM
(the memory roofline), cast in-flight to bf16, computing per-batch-group
partial rowsums on the fly; small [1024]-float AllReduces make the
rowsums global, after which the still-resident W tiles feed the update
matmul (update.T accumulated entirely in PSUM over all 32 batch chunks).

Attention runs transposed (scoresT[slot, batch] = mem_newT.T @ QT) so
softmax stats never need a partition-axis reduction on the hot path:
  - exp(s - 60) with NO per-row max subtraction (constant shift cancels
    in softmax and in maxE/Z; keeps exp in fp32/bf16 range);
    softmax denominator Z comes for free as a ones-column in the
    retrieved matmul; confidence = maxE / Z with maxE tracked by a
    bf16 elementwise-max accumulator (partition-reduced at the end
    via PE transposes). ACT runs ONLY Exp (all PSUM drains are on DVE)
    to avoid per-instruction activation-table reloads.
  - retrieved partials: R[b, f+1] = sum_j E[j,b] * [mem_new | 1][j, f]
    Each R accumulator gets a whole PSUM bank: start=True clears the
    entire bank, so co-resident accumulators would be erased.
Host combines: sums R/Z partials over cores, maxes maxE, concatenates
mem_new slices. Only device collective: the rowsum AllReduces.

Precision: W/X/E/mem_new(bf16) paths ~0.4% RMS; scores in fp32r
(~2.5e-4) since exp amplifies score error.
"""
import sys

if "/opt/trn_rl_repo" not in sys.path:
    sys.path.insert(0, "/opt/trn_rl_repo")

import numpy as np

import concourse.bacc as bacc
import concourse.mybir as mybir
import concourse.tile as tile
import concourse.masks as masks
from concourse.bass_utils import run_bass_kernel_spmd

N_CORES = 8
B = 4096           # batch
F = 128            # feature dim
M = 32768          # memory slots (global)
MS = M // N_CORES  # slots per core = 4096
NB = B // 128      # batch chunks of 128 = 32
NG = 4             # batch groups (rowsum AllReduce granularity)
GK = NB // NG      # chunks per group = 8
SC = MS // 128     # slot chunks per core = 32
AG = 8             # attention batch groups of 512
F32 = mybir.dt.float32
F32R = mybir.dt.float32r
BF16 = mybir.dt.bfloat16

_CACHED_NC = None


def _build():
    nc = bacc.Bacc("TRN2", target_bir_lowering=False, debug=False,
                   num_devices=N_CORES)
    w = nc.dram_tensor("w", [B, MS], F32, kind="ExternalInput").ap()
    qt = nc.dram_tensor("qt", [F, B], F32, kind="ExternalInput").ap()
    x = nc.dram_tensor("x", [B, F], F32, kind="ExternalInput").ap()
    mt = nc.dram_tensor("mt", [F, MS], F32, kind="ExternalInput").ap()

    mem_newt_o = nc.dram_tensor("mem_newt", [F, MS], F32,
                                kind="ExternalOutput").ap()
    r_o = nc.dram_tensor("r_out", [AG, 128, 4 * 129], F32,
                         kind="ExternalOutput").ap()
    maxe_o = nc.dram_tensor("maxe", [128, NB], F32,
                            kind="ExternalOutput").ap()

    rg = [list(range(N_CORES))]

    with tile.TileContext(nc) as tc:
        with (
            tc.tile_pool(name="const", bufs=1) as const,
            tc.tile_pool(name="dram", bufs=16, space="DRAM") as dram,
        ):
            ident = const.tile([128, 128], F32)
            masks.make_identity(nc, ident[:])
            mem_newt_r = const.tile([F, MS], F32R)  # memory + update (f32r)

            # ================= write phase =================
            with (
                tc.tile_pool(name="wconst", bufs=1) as wconst,
                tc.tile_pool(name="wpool", bufs=16) as wpool,
                tc.tile_pool(name="spool", bufs=4) as spool,
                tc.tile_pool(name="psum_u", bufs=1, space="PSUM") as up,
            ):
                # x rearranged to [128, 32*128]: chunk i at cols i*128..,
                # partition p = batch row i*128+p.
                xsb = wconst.tile([128, NB * F], BF16)
                nc.gpsimd.dma_start(
                    out=xsb[:].rearrange("p (c f) -> p c f", c=NB),
                    in_=x.rearrange("(c p) f -> p c f", p=128))
                xs = wconst.tile([128, NB * F], BF16)  # x * (1/rowsum)
                mtt = wconst.tile([F, MS], F32)
                nc.sync.dma_start(out=mtt[:], in_=mt[:, :])

                psum_u = up.tile([F, MS], F32)
                wtiles = {}
                for g in range(NG):
                    sp = spool.tile([128, GK], F32)
                    for k in range(GK):
                        i = g * GK + k
                        wt = wpool.tile([128, MS], BF16, tag="wt")
                        nc.gpsimd.dma_start(
                            out=wt[:], in_=w[i * 128:(i + 1) * 128, :])
                        nc.vector.reduce_sum(out=sp[:, k:k + 1], in_=wt[:],
                                             axis=mybir.AxisListType.X)
                        wtiles[i] = wt
                    cc_in = dram.tile([128 * GK], F32, tag="cc_in")
                    cc_out = dram.tile([128 * GK], F32, tag="cc_out")
                    nc.sync.dma_start(
                        out=cc_in[:].rearrange("(k p) -> p k", p=128),
                        in_=sp[:])
                    nc.gpsimd.collective_compute(
                        "AllReduce", mybir.AluOpType.add, replica_groups=rg,
                        ins=[cc_in[:]], outs=[cc_out[:]])
                    rsg = spool.tile([128, GK], F32)
                    nc.sync.dma_start(
                        out=rsg[:],
                        in_=cc_out[:].rearrange("(k p) -> p k", p=128))
                    rc = spool.tile([128, GK], F32)
                    nc.vector.reciprocal(rc[:], rsg[:])
                    for k in range(GK):
                        i = g * GK + k
                        nc.vector.tensor_scalar_mul(
                            xs[:, i * F:(i + 1) * F],
                            xsb[:, i * F:(i + 1) * F], rc[:, k:k + 1])
                        wt = wtiles.pop(i)
                        for n in range(MS // 512):
                            nc.tensor.matmul(
                                psum_u[:, n * 512:(n + 1) * 512],
                                lhsT=xs[:, i * F:(i + 1) * F],
                                rhs=wt[:, n * 512:(n + 1) * 512],
                                start=(i == 0), stop=(i == NB - 1))

                # mem_newT = update.T + memory.T, rounded to f32r
                nc.vector.tensor_add(mem_newt_r[:], psum_u[:], mtt[:])

            nc.sync.dma_start(out=mem_newt_o[:, :],
                              in_=mem_newt_r[:].bitcast(F32))

            # ================= attention phase =================
            with (
                tc.tile_pool(name="aconst", bufs=1) as aconst,
                tc.tile_pool(name="epool", bufs=3) as epool,
                tc.tile_pool(name="opool", bufs=2) as opool,
                tc.tile_pool(name="psum_s", bufs=2, space="PSUM") as sps,
                tc.tile_pool(name="psum_r", bufs=1, space="PSUM") as rps,
                tc.tile_pool(name="psum_m", bufs=2, space="PSUM") as mps,
            ):
                qtr = aconst.tile([F, B], F32R)
                nc.sync.dma_start(out=qtr[:], in_=qt[:, :].bitcast(F32R))
                mnb = aconst.tile([128, SC * 129], BF16)  # [mem_new | 1]
                macc = aconst.tile([128, B], BF16)        # running max of E
                nc.vector.memset(macc[:], 0.0)
                nc.gpsimd.memset(mnb[:], 1.0)
                mef = aconst.tile([128, B], F32)
                mesb = opool.tile([128, NB], F32)
                nbias = aconst.tile([128, 1], F32)        # exp shift
                nc.vector.memset(nbias[:], -60.0)

                # mnb: [mem_new | 1] per slot chunk, bf16 (via PE transpose)
                for t in range(SC):
                    pt = mps.tile([128, F], F32, tag="pt")
                    nc.tensor.transpose(
                        pt[:], mem_newt_r[:, t * 128:(t + 1) * 128]
                        .bitcast(F32), ident[:])
                    nc.vector.tensor_copy(mnb[:, t * 129:t * 129 + 128],
                                          pt[:])

                # Keep PE transpose-mode ops strictly before the attention
                # Ldweights/Matmult stream.
                tc.no_sync_barrier()

                for g in range(AG):
                    rts = []
                    for bt in range(4):
                        rtile = rps.tile([128, 129], F32, tag=f"r{bt}")
                        rts.append(rtile)
                    for t in range(SC):
                        st = sps.tile([128, 512], F32, tag="st")
                        nc.tensor.matmul(
                            st[:], lhsT=mem_newt_r[:, t * 128:(t + 1) * 128],
                            rhs=qtr[:, g * 512:(g + 1) * 512],
                            start=True, stop=True)
                        et = epool.tile([128, 512], BF16, tag="et")
                        nc.scalar.activation(et[:], st[:],
                                             mybir.ActivationFunctionType.Exp,
                                             bias=nbias[:])
                        nc.vector.tensor_max(macc[:, g * 512:(g + 1) * 512],
                                             macc[:, g * 512:(g + 1) * 512],
                                             et[:])
                        for bt in range(4):
                            nc.tensor.matmul(
                                rts[bt][:, :],
                                lhsT=et[:, bt * 128:(bt + 1) * 128],
                                rhs=mnb[:, t * 129:(t + 1) * 129],
                                start=(t == 0), stop=(t == SC - 1))
                    rsb = opool.tile([128, 4 * 129], F32, tag="rsb")
                    for bt in range(4):
                        nc.vector.tensor_copy(
                            rsb[:, bt * 129:(bt + 1) * 129], rts[bt][:])
                    nc.sync.dma_start(out=r_o[g, :, :], in_=rsb[:])

                # confidence: partition-reduce macc via PE transposes
                tc.no_sync_barrier()
                nc.vector.tensor_copy(mef[:], macc[:])
                for t in range(NB):
                    pt2 = mps.tile([128, 128], F32, tag="pt")
                    nc.tensor.transpose(pt2[:], mef[:, t * 128:(t + 1) * 128],
                                        ident[:])
                    nc.vector.reduce_max(out=mesb[:, t:t + 1], in_=pt2[:],
                                         axis=mybir.AxisListType.X)
                nc.sync.dma_start(out=maxe_o[:, :], in_=mesb[:])

    nc.compile()
    return nc


def kernel(memory, input_data, write_weights, query):
    global _CACHED_NC
    memory = np.asarray(memory, dtype=np.float32)
    input_data = np.asarray(input_data, dtype=np.float32)
    write_weights = np.asarray(write_weights, dtype=np.float32)
    query = np.asarray(query, dtype=np.float32)

    if _CACHED_NC is None:
        _CACHED_NC = _build()
    nc = _CACHED_NC

    qt = np.ascontiguousarray(query.T)
    in_maps = []
    for c in range(N_CORES):
        in_maps.append({
            "w": np.ascontiguousarray(write_weights[:, c * MS:(c + 1) * MS]),
            "qt": qt,
            "x": input_data,
            "mt": np.ascontiguousarray(memory[c * MS:(c + 1) * MS, :].T),
        })

    res = run_bass_kernel_spmd(nc, in_maps, list(range(N_CORES))).results

    # ---------- host combine (cheap: ~2MB/core) ----------
    mem_new = np.concatenate(
        [res[c]["mem_newt"].T for c in range(N_CORES)], axis=0)

    R = np.zeros((B, 129), dtype=np.float64)
    maxe = np.zeros(B, dtype=np.float64)
    for c in range(N_CORES):
        r4 = res[c]["r_out"].reshape(AG, 128, 4, 129)  # [g, p, bt, col]
        R += r4.transpose(0, 2, 1, 3).reshape(B, 129)
        me = res[c]["maxe"].T.reshape(B)               # b = t*128 + p
        maxe = np.maximum(maxe, me)

    z = R[:, 128]
    retrieved = (R[:, :F] / z[:, None]).astype(np.float32)
    conf = (maxe / z).astype(np.float32)
    return retrieved, conf, np.ascontiguousarray(mem_new)


# revision 17
# speedup vs baseline: 1.1625x; 1.0018x over previous
"""MemoryBank kernel for Trainium2, 8 NeuronCores (SPMD).

Reference computation:
    s        = rowsum(W)                       # [B]
    update   = (W/s[:,None]).T @ X             # [M, F]
    mem_new  = memory + update
    scores   = Q @ mem_new.T                   # [B, M]
    attn     = softmax(scores, axis=1)
    retrieved = attn @ mem_new                 # [B, F]
    conf     = max(attn, axis=1)               # [B]

Sharding: memory_size axis. Core c owns slots [c*4096, (c+1)*4096).
Each core reads its column slice of W (64 MB) EXACTLY ONCE from HBM
(the memory roofline), cast in-flight to bf16, computing per-batch-group
partial rowsums on the fly; small [1024]-float AllReduces make the
rowsums global, after which the still-resident W tiles feed the update
matmul (update.T accumulated entirely in PSUM over all 32 batch chunks).

Attention runs transposed (scoresT[slot, batch] = mem_newT.T @ QT) so
softmax stats never need a partition-axis reduction on the hot path:
  - exp(s - 60) with NO per-row max subtraction (constant shift cancels
    in softmax and in maxE/Z; keeps exp in fp32/bf16 range);
    softmax denominator Z comes for free as a ones-column in the
    retrieved matmul; confidence = maxE / Z with maxE tracked by a
    bf16 elementwise-max accumulator (partition-reduced at the end
    via PE transposes). ACT runs ONLY Exp (all PSUM drains are on DVE)
    to avoid per-instruction activation-table reloads.
  - retrieved partials: R[b, f+1] = sum_j E[j,b] * [mem_new | 1][j, f]
    Each R accumulator gets a whole PSUM bank: start=True clears the
    entire bank, so co-resident accumulators would be erased.
Host combines: sums R/Z partials over cores, maxes maxE, concatenates
mem_new slices. Only device collective: the rowsum AllReduces.

Precision: W/X/E/mem_new(bf16) paths ~0.4% RMS; scores in fp32r
(~2.5e-4) since exp amplifies score error.
"""
import sys

if "/opt/trn_rl_repo" not in sys.path:
    sys.path.insert(0, "/opt/trn_rl_repo")

import numpy as np

import concourse.bacc as bacc
import concourse.mybir as mybir
import concourse.tile as tile
import concourse.masks as masks
from concourse.bass_utils import run_bass_kernel_spmd

N_CORES = 8
B = 4096           # batch
F = 128            # feature dim
M = 32768          # memory slots (global)
MS = M // N_CORES  # slots per core = 4096
NB = B // 128      # batch chunks of 128 = 32
NG = 4             # batch groups (rowsum AllReduce granularity)
GK = NB // NG      # chunks per group = 8
SC = MS // 128     # slot chunks per core = 32
AG = 8             # attention batch groups of 512
F32 = mybir.dt.float32
F32R = mybir.dt.float32r
BF16 = mybir.dt.bfloat16

_CACHED_NC = None


def _build():
    nc = bacc.Bacc("TRN2", target_bir_lowering=False, debug=False,
                   num_devices=N_CORES)
    w = nc.dram_tensor("w", [B, MS], F32, kind="ExternalInput").ap()
    qt = nc.dram_tensor("qt", [F, B], F32, kind="ExternalInput").ap()
    x = nc.dram_tensor("x", [B, F], F32, kind="ExternalInput").ap()
    mt = nc.dram_tensor("mt", [F, MS], F32, kind="ExternalInput").ap()

    mem_newt_o = nc.dram_tensor("mem_newt", [F, MS], F32,
                                kind="ExternalOutput").ap()
    r_o = nc.dram_tensor("r_out", [AG, 128, 4 * 129], F32,
                         kind="ExternalOutput").ap()
    maxe_o = nc.dram_tensor("maxe", [128, NB], F32,
                            kind="ExternalOutput").ap()

    rg = [list(range(N_CORES))]

    with tile.TileContext(nc) as tc:
        with (
            tc.tile_pool(name="const", bufs=1) as const,
            tc.tile_pool(name="dram", bufs=16, space="DRAM") as dram,
        ):
            ident = const.tile([128, 128], F32)
            masks.make_identity(nc, ident[:])
            mem_newt_r = const.tile([F, MS], F32R)  # memory + update (f32r)

            # ================= write phase =================
            with (
                tc.tile_pool(name="wconst", bufs=1) as wconst,
                tc.tile_pool(name="wpool", bufs=16) as wpool,
                tc.tile_pool(name="spool", bufs=4) as spool,
                tc.tile_pool(name="psum_u", bufs=1, space="PSUM") as up,
            ):
                # x rearranged to [128, 32*128]: chunk i at cols i*128..,
                # partition p = batch row i*128+p.
                xsb = wconst.tile([128, NB * F], BF16)
                nc.gpsimd.dma_start(
                    out=xsb[:].rearrange("p (c f) -> p c f", c=NB),
                    in_=x.rearrange("(c p) f -> p c f", p=128))
                xs = wconst.tile([128, NB * F], BF16)  # x * (1/rowsum)
                mtt = wconst.tile([F, MS], F32)
                nc.sync.dma_start(out=mtt[:], in_=mt[:, :])

                psum_u = up.tile([F, MS], F32)
                wtiles = {}
                for g in range(NG):
                    sp = spool.tile([128, GK], F32)
                    for k in range(GK):
                        i = g * GK + k
                        wt = wpool.tile([128, MS], BF16, tag="wt")
                        nc.gpsimd.dma_start(
                            out=wt[:], in_=w[i * 128:(i + 1) * 128, :])
                        nc.vector.reduce_sum(out=sp[:, k:k + 1], in_=wt[:],
                                             axis=mybir.AxisListType.X)
                        wtiles[i] = wt
                    cc_in = dram.tile([128 * GK], F32, tag="cc_in")
                    cc_out = dram.tile([128 * GK], F32, tag="cc_out")
                    nc.sync.dma_start(
                        out=cc_in[:].rearrange("(k p) -> p k", p=128),
                        in_=sp[:])
                    nc.gpsimd.collective_compute(
                        "AllReduce", mybir.AluOpType.add, replica_groups=rg,
                        ins=[cc_in[:]], outs=[cc_out[:]])
                    rsg = spool.tile([128, GK], F32)
                    nc.sync.dma_start(
                        out=rsg[:],
                        in_=cc_out[:].rearrange("(k p) -> p k", p=128))
                    rc = spool.tile([128, GK], F32)
                    nc.vector.reciprocal(rc[:], rsg[:])
                    for k in range(GK):
                        i = g * GK + k
                        nc.vector.tensor_scalar_mul(
                            xs[:, i * F:(i + 1) * F],
                            xsb[:, i * F:(i + 1) * F], rc[:, k:k + 1])
                        wt = wtiles.pop(i)
                        for n in range(MS // 512):
                            nc.tensor.matmul(
                                psum_u[:, n * 512:(n + 1) * 512],
                                lhsT=xs[:, i * F:(i + 1) * F],
                                rhs=wt[:, n * 512:(n + 1) * 512],
                                start=(i == 0), stop=(i == NB - 1))

                # mem_newT = update.T + memory.T, rounded to f32r
                nc.vector.tensor_add(mem_newt_r[:], psum_u[:], mtt[:])

            nc.sync.dma_start(out=mem_newt_o[:, :],
                              in_=mem_newt_r[:].bitcast(F32))

            # ================= attention phase =================
            with (
                tc.tile_pool(name="aconst", bufs=1) as aconst,
                tc.tile_pool(name="epool", bufs=4) as epool,
                tc.tile_pool(name="opool", bufs=2) as opool,
                tc.tile_pool(name="psum_s", bufs=4, space="PSUM") as sps,
                tc.tile_pool(name="psum_r", bufs=1, space="PSUM") as rps,
            ):
                qtr = aconst.tile([F, B], F32R)
                nc.sync.dma_start(out=qtr[:], in_=qt[:, :].bitcast(F32R))
                mnb = aconst.tile([128, SC * 129], BF16)  # [mem_new | 1]
                macc = aconst.tile([128, B], BF16)        # running max of E
                nc.vector.memset(macc[:], 0.0)
                nc.gpsimd.memset(mnb[:], 1.0)
                mef = aconst.tile([128, B], F32)
                mesb = opool.tile([128, NB], F32)
                nbias = aconst.tile([128, 1], F32)        # exp shift
                nc.vector.memset(nbias[:], -60.0)

                # mnb: [mem_new | 1] per slot chunk, bf16 (via PE transpose).
                # Transpose tiles share the st pool slots (same tag) to keep
                # the PSUM budget at 8 banks with 4-deep st buffering.
                for t in range(SC):
                    pt = sps.tile([128, 512], F32, tag="st")
                    nc.tensor.transpose(
                        pt[:, :F], mem_newt_r[:, t * 128:(t + 1) * 128]
                        .bitcast(F32), ident[:])
                    nc.vector.tensor_copy(mnb[:, t * 129:t * 129 + 128],
                                          pt[:, :F])

                # Keep PE transpose-mode ops strictly before the attention
                # Ldweights/Matmult stream.
                tc.no_sync_barrier()

                for g in range(AG):
                    rts = []
                    for bt in range(4):
                        rtile = rps.tile([128, 129], F32, tag=f"r{bt}")
                        rts.append(rtile)
                    for t in range(SC):
                        st = sps.tile([128, 512], F32, tag="st")
                        nc.tensor.matmul(
                            st[:], lhsT=mem_newt_r[:, t * 128:(t + 1) * 128],
                            rhs=qtr[:, g * 512:(g + 1) * 512],
                            start=True, stop=True)
                        et = epool.tile([128, 512], BF16, tag="et")
                        nc.scalar.activation(et[:], st[:],
                                             mybir.ActivationFunctionType.Exp,
                                             bias=nbias[:])
                        nc.vector.tensor_max(macc[:, g * 512:(g + 1) * 512],
                                             macc[:, g * 512:(g + 1) * 512],
                                             et[:])
                        for bt in range(4):
                            nc.tensor.matmul(
                                rts[bt][:, :],
                                lhsT=et[:, bt * 128:(bt + 1) * 128],
                                rhs=mnb[:, t * 129:(t + 1) * 129],
                                start=(t == 0), stop=(t == SC - 1))
                    rsb = opool.tile([128, 4 * 129], F32, tag="rsb")
                    for bt in range(4):
                        nc.vector.tensor_copy(
                            rsb[:, bt * 129:(bt + 1) * 129], rts[bt][:])
                    nc.sync.dma_start(out=r_o[g, :, :], in_=rsb[:])

                # confidence: partition-reduce macc via PE transposes
                tc.no_sync_barrier()
                nc.vector.tensor_copy(mef[:], macc[:])
                for t in range(NB):
                    pt2 = sps.tile([128, 512], F32, tag="st")
                    nc.tensor.transpose(pt2[:, :128],
                                        mef[:, t * 128:(t + 1) * 128],
                                        ident[:])
                    nc.vector.reduce_max(out=mesb[:, t:t + 1],
                                         in_=pt2[:, :128],
                                         axis=mybir.AxisListType.X)
                nc.sync.dma_start(out=maxe_o[:, :], in_=mesb[:])

    nc.compile()
    return nc


def kernel(memory, input_data, write_weights, query):
    global _CACHED_NC
    memory = np.asarray(memory, dtype=np.float32)
    input_data = np.asarray(input_data, dtype=np.float32)
    write_weights = np.asarray(write_weights, dtype=np.float32)
    query = np.asarray(query, dtype=np.float32)

    if _CACHED_NC is None:
        _CACHED_NC = _build()
    nc = _CACHED_NC

    qt = np.ascontiguousarray(query.T)
    in_maps = []
    for c in range(N_CORES):
        in_maps.append({
            "w": np.ascontiguousarray(write_weights[:, c * MS:(c + 1) * MS]),
            "qt": qt,
            "x": input_data,
            "mt": np.ascontiguousarray(memory[c * MS:(c + 1) * MS, :].T),
        })

    res = run_bass_kernel_spmd(nc, in_maps, list(range(N_CORES))).results

    # ---------- host combine (cheap: ~2MB/core) ----------
    mem_new = np.concatenate(
        [res[c]["mem_newt"].T for c in range(N_CORES)], axis=0)

    R = np.zeros((B, 129), dtype=np.float64)
    maxe = np.zeros(B, dtype=np.float64)
    for c in range(N_CORES):
        r4 = res[c]["r_out"].reshape(AG, 128, 4, 129)  # [g, p, bt, col]
        R += r4.transpose(0, 2, 1, 3).reshape(B, 129)
        me = res[c]["maxe"].T.reshape(B)               # b = t*128 + p
        maxe = np.maximum(maxe, me)

    z = R[:, 128]
    retrieved = (R[:, :F] / z[:, None]).astype(np.float32)
    conf = (maxe / z).astype(np.float32)
    return retrieved, conf, np.ascontiguousarray(mem_new)
